# revision 15
# baseline (speedup 1.0000x reference)
"""Trainium2 Bass kernel: ActiveBlockPromptBasis (moe_routing).

Math (per batch image b):
  g   = gelu(W1x @ x_b + b1x  ++  W1t @ flux_b + b1t)        # [14, pix]
  z   = Mz.T @ g + bz          (fc2 of both MLPs fused with the 6x8
                                outer-sum expansion, in log space)  # [48, pix]
  wun = exp(z);  S = colsum(wun);  w = wun / S                # exact softmax
  P   = prompt_flat.T @ w                                     # [128, pix]
  out = conv3x3(P, conv_w)     (9 accumulating matmuls / 2-row PSUM bank)

Sharding: data-parallel over batch, one image per NeuronCore (8 cores).
"""

import numpy as np
from collections import deque
from contextlib import ExitStack

import concourse.bass as bass
import concourse.tile as tile
from concourse import bacc, mybir
from concourse.bass_utils import run_bass_kernel_spmd

F32 = mybir.dt.float32
AFT = mybir.ActivationFunctionType

B, DIM, E = 8, 64, 128
NT, NB = 6, 8
NTK = NT * NB  # 48
NCORES = 8


def build_program(h=256, w=256, r_out=32, gelu_fn=AFT.Gelu, mm_dt="f32r",
                  external_io=True, repeat=1):
    """Build the single-core Bass program (SPMD: same program on all cores).

    mm_dt: "f32" (exact, 1/4-rate PE) or "f32r" (full-rate PE, ~tf32-ish
    products with fp32 accumulate).
    """
    assert w == 256 and r_out % 8 == 0 and h % r_out == 0
    PIX = h * w

    MDT = mybir.dt.float32r if mm_dt == "f32r" else F32
    PITCH = w + 2          # row window with 1 zero spacer col each side
    IR_MAX = r_out + 2     # input rows per strip incl halo

    nc = bacc.Bacc("TRN2", target_bir_lowering=False, debug=False,
                   enable_asserts=False)

    # --- DRAM I/O (per-core slices / replicated small weights) ---
    kin = "ExternalInput" if external_io else "Internal"
    kout = "ExternalOutput" if external_io else "Internal"
    xf_d = nc.dram_tensor("xf", [128, PIX], MDT, kind=kin)
    fb_d = nc.dram_tensor("fb", [65, PIX], MDT, kind=kin)
    wa_d = nc.dram_tensor("wa", [128, 14], MDT, kind=kin)
    wb_d = nc.dram_tensor("wb", [65, 14], MDT, kind=kin)
    mz_d = nc.dram_tensor("mz", [14, NTK], MDT, kind=kin)
    bz_d = nc.dram_tensor("bz", [NTK, 1], F32, kind=kin)
    on48_d = nc.dram_tensor("on48", [NTK, 1], MDT, kind=kin)
    on1_d = nc.dram_tensor("on1", [1, NTK], MDT, kind=kin)
    pt_d = nc.dram_tensor("pt", [NTK, E], MDT, kind=kin)
    wt_d = nc.dram_tensor("wt", [9, E, E], MDT, kind=kin)
    out_d = nc.dram_tensor("out", [E, PIX], F32, kind=kout)
    if not external_io:
        outs_d = nc.dram_tensor("outs", [1, 8], F32, kind="ExternalOutput")

    with tile.TileContext(nc) as tc, ExitStack() as ctx:
        consts = ctx.enter_context(tc.tile_pool(name="consts", bufs=1))
        pin = ctx.enter_context(tc.tile_pool(name="pin", bufs=6))
        psb = ctx.enter_context(tc.tile_pool(name="psb", bufs=6))
        pg_pool = ctx.enter_context(tc.tile_pool(name="pg", bufs=IR_MAX // 2 + 2))
        ppool = ctx.enter_context(
            tc.tile_pool(name="ppsum", bufs=8, space="PSUM"))
        pP = ctx.enter_context(tc.tile_pool(name="pP", bufs=2))
        pout = ctx.enter_context(tc.tile_pool(name="pout", bufs=2))

        # --- load constants once ---
        wa_sb = consts.tile([128, 14], MDT)
        nc.sync.dma_start(out=wa_sb[:], in_=wa_d[:])
        wb_sb = consts.tile([65, 14], MDT)
        nc.sync.dma_start(out=wb_sb[:], in_=wb_d[:])
        mz_sb = consts.tile([14, NTK], MDT)
        nc.sync.dma_start(out=mz_sb[:], in_=mz_d[:])
        bz_sb = consts.tile([NTK, 1], F32)
        nc.sync.dma_start(out=bz_sb[:], in_=bz_d[:])
        on48_sb = consts.tile([NTK, 1], MDT)
        nc.sync.dma_start(out=on48_sb[:], in_=on48_d[:])
        on1_sb = consts.tile([1, NTK], MDT)
        nc.sync.dma_start(out=on1_sb[:], in_=on1_d[:])
        pt_sb = consts.tile([NTK, E], MDT)
        nc.sync.dma_start(out=pt_sb[:], in_=pt_d[:])
        wt_sb = consts.tile([E, 9 * E], MDT)
        for t in range(9):
            nc.sync.dma_start(out=wt_sb[:, t * E:(t + 1) * E], in_=wt_d[t])

        n_strips = h // r_out
        conv_queue = deque()
        cur_out = {"t": None}
        last_exp_inst = None

        def emit_conv_pair(item):
            cP3, cr0, yA, drain_sel = item
            pcv = ppool.tile([128, 512], F32, tag="bank")
            taps = []
            for ky in (1, 0, 2):
                rlo, rhi = yA, yA + 1
                if yA + ky - 1 < 0:
                    rlo = yA + 1
                if yA + 1 + ky - 1 > h - 1:
                    rhi = yA
                for kx in (0, 1, 2):
                    taps.append((ky, kx, rlo, rhi))
            for ti, (ky, kx, rlo, rhi) in enumerate(taps):
                nr = rhi - rlo + 1
                lr = rlo + ky - 1 - cr0
                tap = ky * 3 + kx
                nc.tensor.matmul(
                    pcv[:, (rlo - yA) * w:(rhi - yA + 1) * w],
                    wt_sb[:, tap * E:(tap + 1) * E],
                    cP3[:, lr:lr + nr, kx:kx + w],
                    start=(ti == 0), stop=(ti == len(taps) - 1))
            # drain into the 8-row staging buffer; DMA when full
            q = (yA // 2) % 4
            if q == 0:
                cur_out["t"] = pout.tile([128, 2048], F32, tag="outsb",
                                         name="outsb")
            dst = cur_out["t"][:, q * 512:(q + 1) * 512]
            if drain_sel:
                nc.vector.tensor_copy(dst, pcv[:])
            else:
                nc.scalar.copy(dst, pcv[:])
            if q == 3:
                g0 = yA - 6
                nc.sync.dma_start(out=out_d[:, g0 * w:(g0 + 8) * w],
                                  in_=cur_out["t"][:])

        def emit_body():
            nonlocal last_exp_inst
            last_exp_inst = None
            for s in range(n_strips):
                emit_strip(s)
            while conv_queue:
                emit_conv_pair(conv_queue.popleft())

        def emit_strip(s):
            nonlocal last_exp_inst
            y0, y1 = s * r_out, (s + 1) * r_out
            r0, r1 = max(0, y0 - 1), min(h - 1, y1)  # input rows incl halo
            ir = r1 - r0 + 1

            P_t = pP.tile([128, IR_MAX * PITCH], MDT, tag="P")
            P3 = P_t[:].rearrange("p (r c) -> p r c", c=PITCH)
            # zero the spacer columns (left/right zero padding for the conv)
            nc.vector.memset(P3[:, :ir, 0:1].bitcast(F32), 0.0)
            nc.vector.memset(P3[:, :ir, PITCH - 1:PITCH].bitcast(F32), 0.0)

            chunks = []
            r = r0
            while r <= r1:
                nrows = min(2, r1 - r + 1)
                chunks.append((r, nrows))
                r += nrows

            # ---- stage 1 phase A: fc1 + gelu (one ACT table set) ----
            g_tiles = []
            first_gelu_inst = None
            for (r, nrows) in chunks:
                npix = nrows * w
                off = r * w
                xf_t = pin.tile([128, 512], MDT, tag="xf")
                nc.sync.dma_start(out=xf_t[:, :npix], in_=xf_d[:, off:off + npix])
                fb_t = pin.tile([65, 512], MDT, tag="fb")
                nc.scalar.dma_start(out=fb_t[:, :npix], in_=fb_d[:, off:off + npix])
                pg = ppool.tile([14, 512], F32, tag="bank")
                nc.tensor.matmul(pg[:, :npix], wa_sb[:], xf_t[:, :npix],
                                 start=True, stop=False)
                nc.tensor.matmul(pg[:, :npix], wb_sb[:], fb_t[:, :npix],
                                 start=False, stop=True)
                g_t = pg_pool.tile([14, 512], MDT, tag="g")
                inst = nc.scalar.activation(g_t[:, :npix], pg[:, :npix], gelu_fn)
                if first_gelu_inst is None:
                    first_gelu_inst = inst
                g_tiles.append(g_t)
                if conv_queue and conv_queue[0][0] is not P3:
                    emit_conv_pair(conv_queue.popleft())

            # keep ACT-engine phases ordered across strips so walrus doesn't
            # re-load activation tables on interleaved gelu/exp runs
            if last_exp_inst is not None and first_gelu_inst is not None:
                bass._add_dep_helper(first_gelu_inst.ins, last_exp_inst.ins,
                                     sync=True, reason="act-table-phase-order")

            # ---- stage 1 phase B + conv, software-pipelined emission ----
            # Per-engine issue is in-order, so a chunk-major relay chain
            # (mmZ -> exp -> mmS -> recip -> mmB -> mul -> mmP -> copy)
            # serializes the machine.  Emit stage k of chunk i alongside
            # stage k+1 of chunk i-1 etc., and interleave ready conv pairs
            # (no pending deps) as PE gap-filler.
            NCH = len(chunks)

            def stage0(ci):   # fc2+expand matmul, exp
                nonlocal last_exp_inst
                r, nrows = chunks[ci]
                npix = nrows * w
                pzs = ppool.tile([NTK, 512], F32, tag="bank")
                st["pzs"][ci] = pzs
                nc.tensor.matmul(pzs[0:NTK, :npix], mz_sb[:],
                                 g_tiles[ci][:, :npix])
                wun = psb.tile([NTK, 512], MDT, tag="wun")
                st["wun"][ci] = wun
                last_exp_inst = nc.scalar.activation(
                    wun[:, :npix], pzs[0:NTK, :npix], AFT.Exp, bias=bz_sb[:])

            def stage1(ci):   # colsum, reciprocal
                r, nrows = chunks[ci]
                npix = nrows * w
                ps_t = ppool.tile([1, 512], F32, tag="bank", name="ps_t")
                st["ps"][ci] = ps_t
                nc.tensor.matmul(ps_t[:, :npix], on48_sb[:],
                                 st["wun"][ci][:, :npix])
                rr = psb.tile([1, 512], MDT, tag="rr")
                st["rr"][ci] = rr
                if mm_dt == "f32r":
                    from concourse.dve_ops import (RECIP_APPROX_FAST_CONSTS,
                                                   RECIPROCAL_APPROX_FAST)
                    c = RECIP_APPROX_FAST_CONSTS
                    nc.vector._custom_dve(
                        RECIPROCAL_APPROX_FAST, out=rr[:, :npix],
                        in0=st["ps"][ci][:, :npix],
                        s0=c["s0"], s1=c["s1"], imm2=c["imm2"])
                else:
                    nc.vector.reciprocal_approx_fast(
                        rr[:, :npix], st["ps"][ci][:, :npix])

            def stage2(ci):   # broadcast 1/S, normalize weights
                r, nrows = chunks[ci]
                npix = nrows * w
                prb = ppool.tile([NTK, 512], F32, tag="bank")
                nc.tensor.matmul(prb[:, :npix], on1_sb[:],
                                 st["rr"][ci][:, :npix])
                w_t = psb.tile([NTK, 512], MDT, tag="w")
                st["w"][ci] = w_t
                nc.vector.tensor_mul(w_t[:, :npix], st["wun"][ci][:, :npix],
                                     prb[:, :npix])

            def stage3(ci):   # prompt matmul, copy into P strip
                r, nrows = chunks[ci]
                npix = nrows * w
                pp = ppool.tile([128, 512], F32, tag="bank")
                nc.tensor.matmul(pp[:, :npix], pt_sb[:],
                                 st["w"][ci][:, :npix])
                lr = r - r0
                nc.vector.tensor_copy(
                    P3[:, lr:lr + nrows, 1:1 + w],
                    pp[:, :npix].rearrange("p (r c) -> p r c", c=w))

            st = {"pzs": {}, "ps": {}, "wun": {}, "rr": {}, "w": {}}

            # queue this strip's conv pairs: pair (yA) becomes emittable one
            # iteration after stage3 of chunk covering its last input row
            for pi in range((y1 - y0) // 2):
                conv_queue.append((P3, r0, y0 + 2 * pi, pi % 2))

            for it in range(NCH + 4):
                if it < NCH:
                    stage0(it)
                if 0 <= it - 1 < NCH:
                    stage1(it - 1)
                if 0 <= it - 2 < NCH:
                    stage2(it - 2)
                if 0 <= it - 3 < NCH:
                    stage3(it - 3)
                # conv pair p of THIS strip needs stage3 of chunks <= p+1,
                # i.e. emitted at iteration >= p+5; older strips' leftovers
                # are always ready
                while conv_queue:
                    cP3, cr0, yA, dsel = conv_queue[0]
                    if cP3 is P3 and it < (yA - y0) // 2 + 5:
                        break
                    if len(conv_queue) <= 6 and s < n_strips - 1:
                        break  # reserve filler pairs for next strip's phase A
                    emit_conv_pair(conv_queue.popleft())
                    break  # at most one pair per iteration

        if repeat > 1:
            with tc.For_i(0, repeat, 1):
                emit_body()
        else:
            emit_body()
        if not external_io:
            nc.sync.dma_start(out=outs_d[:], in_=out_d[0:1, 0:8])

    nc.compile()
    return nc


_cache = {}


def get_program(h=256, w=256, r_out=32, gelu_fn=AFT.Gelu):
    key = (h, w, r_out, gelu_fn)
    if key not in _cache:
        _cache[key] = build_program(h, w, r_out, gelu_fn)
    return _cache[key]


def make_weight_inputs(prompt, conv_w, b_fc1_w, b_fc1_b, b_fc2_w, b_fc2_b,
                       t_fc1_w, t_fc1_b, t_fc2_w, t_fc2_b):
    f = np.float32
    wa = np.zeros((128, 14), f)
    wa[:64, :8] = b_fc1_w.T
    wa[64:128, 8:14] = t_fc1_w[:, :64].T
    wb = np.zeros((65, 14), f)
    wb[:64, 8:14] = t_fc1_w[:, 64:].T
    wb[64, :8] = b_fc1_b
    wb[64, 8:14] = t_fc1_b
    mz = np.zeros((14, NTK), f)
    bz = np.zeros((NTK, 1), f)
    for t in range(NT):
        for k in range(NB):
            c = t * NB + k
            mz[:8, c] = b_fc2_w[k, :]
            mz[8:, c] = t_fc2_w[t, :]
            bz[c, 0] = b_fc2_b[k] + t_fc2_b[t]
    return {
        "wa": wa,
        "wb": wb,
        "mz": mz,
        "bz": bz,
        "on48": np.ones((NTK, 1), f),
        "on1": np.ones((1, NTK), f),
        "pt": np.ascontiguousarray(prompt.reshape(NTK, E).astype(f)),
        "wt": np.ascontiguousarray(
            conv_w.transpose(2, 3, 1, 0).reshape(9, E, E).astype(f)),
    }


def make_core_inputs(x_b, flux_b, weights, h, w):
    PIX = h * w
    f = np.float32
    xf = np.concatenate(
        [x_b.reshape(DIM, PIX), flux_b[:64].reshape(64, PIX)], axis=0)
    fb = np.concatenate(
        [flux_b[64:].reshape(64, PIX), np.ones((1, PIX), f)], axis=0)
    m = {"xf": np.ascontiguousarray(xf, dtype=f),
         "fb": np.ascontiguousarray(fb, dtype=f)}
    m.update(weights)
    return m


def kernel(x, flux, prompt, conv_w, b_fc1_w, b_fc1_b, b_fc2_w, b_fc2_b,
           t_fc1_w, t_fc1_b, t_fc2_w, t_fc2_b):
    x = np.asarray(x, np.float32)
    flux = np.asarray(flux, np.float32)
    flux = np.where(np.isnan(flux), np.float32(0), flux)
    h, w = x.shape[2], x.shape[3]

    nc = get_program(h=h, w=w)
    weights = make_weight_inputs(
        np.asarray(prompt, np.float32), np.asarray(conv_w, np.float32),
        np.asarray(b_fc1_w, np.float32), np.asarray(b_fc1_b, np.float32),
        np.asarray(b_fc2_w, np.float32), np.asarray(b_fc2_b, np.float32),
        np.asarray(t_fc1_w, np.float32), np.asarray(t_fc1_b, np.float32),
        np.asarray(t_fc2_w, np.float32), np.asarray(t_fc2_b, np.float32))
    in_maps = [make_core_inputs(x[i], flux[i], weights, h, w)
               for i in range(NCORES)]
    res = run_bass_kernel_spmd(nc, in_maps, list(range(NCORES)))
    out = np.stack([res.results[i]["out"].reshape(E, h, w)
                    for i in range(NCORES)], axis=0)
    return out


# revision 37
# speedup vs baseline: 31623.1733x; 31623.1733x over previous
"""Trainium2 Bass kernel: ActiveBlockPromptBasis (moe_routing).

Math (per batch image b):
  g   = gelu(W1x @ x_b + b1x  ++  W1t @ flux_b + b1t)        # [14, pix]
  z   = Mz.T @ g + bz          (fc2 of both MLPs fused with the 6x8
                                outer-sum expansion, in log space)  # [48, pix]
  wun = exp(z);  S = colsum(wun)                              # exact softmax
  U   = prompt_flat.T @ wun                                   # [128, pix]
  P   = U * broadcast(1/S)     (approx-recip on DVE; 1/S replicated to all
                                128 partitions by a DRAM-bounce DMA; the
                                multiply is fused into the P-strip write)
  out = conv3x3(P, conv_w)     (9 accumulating f32r matmuls / 2-row PSUM
                                bank; zero spacer columns in the P strip
                                implement the left/right zero padding)

Implementation notes:
  - All matmuls use float32r (tf32-like products, fp32 accumulate): ~2
    cols/cycle on the PE vs 1/4 rate for plain fp32.  End-to-end relative
    error vs the fp32 reference is ~3e-4.
  - Engines issue in order, so the per-chunk softmax relay chain is
    software-pipelined at emission: stage k of chunk i is emitted next to
    stage k+1 of chunk i-1, with always-ready conv matmul pairs
    interleaved as PE gap filler (the conv trails stage 1 by one strip
    via a global ready queue).
  - Gelu and Exp live in different ACT table sets (~2.7us per reload), so
    each strip runs a gelu-only phase then an exp-only phase, with an
    explicit cross-strip ACT ordering dep.
  - DMA traffic is spread across both HWDGE queues (qSP/qAct alternating
    for the 1/S broadcasts, xf on qSP, fb on qAct) and SWDGE (output).

Sharding: data-parallel over batch, one image per NeuronCore (8 cores).
"""

import numpy as np
from collections import deque
from contextlib import ExitStack

import concourse.bass as bass
import concourse.tile as tile
from concourse import bacc, mybir
from concourse.bass_utils import run_bass_kernel_spmd

F32 = mybir.dt.float32
AFT = mybir.ActivationFunctionType

B, DIM, E = 8, 64, 128
NT, NB = 6, 8
NTK = NT * NB  # 48
NCORES = 8


def build_program(h=256, w=256, r_out=32, gelu_fn=AFT.Gelu, mm_dt="f32r",
                  conv_dt=None, external_io=True, repeat=1):
    """Build the single-core Bass program (SPMD: same program on all cores).

    mm_dt: "f32" (exact, 1/4-rate PE) or "f32r" (full-rate PE, ~tf32-ish
    products with fp32 accumulate).
    """
    assert w == 256 and r_out % 8 == 0 and h % r_out == 0
    PIX = h * w

    MDT = mybir.dt.float32r if mm_dt == "f32r" else F32
    CDT = mybir.dt.bfloat16 if conv_dt == "bf16" else MDT
    ACT_DEP = True
    PITCH = w + 2          # row window with 1 zero spacer col each side
    IR_MAX = r_out + 2     # input rows per strip incl halo

    nc = bacc.Bacc("TRN2", target_bir_lowering=False, debug=False,
                   enable_asserts=False)

    # --- DRAM I/O (per-core slices / replicated small weights) ---
    kin = "ExternalInput" if external_io else "Internal"
    kout = "ExternalOutput" if external_io else "Internal"
    xf_d = nc.dram_tensor("xf", [128, PIX], MDT, kind=kin)
    fb_d = nc.dram_tensor("fb", [65, PIX], MDT, kind=kin)
    wa_d = nc.dram_tensor("wa", [128, 14], MDT, kind=kin)
    wb_d = nc.dram_tensor("wb", [65, 14], MDT, kind=kin)
    mz_d = nc.dram_tensor("mz", [14, NTK], MDT, kind=kin)
    bz_d = nc.dram_tensor("bz", [NTK, 1], F32, kind=kin)
    on48_d = nc.dram_tensor("on48", [NTK, 1], MDT, kind=kin)
    on1_d = nc.dram_tensor("on1", [1, NTK], MDT, kind=kin)
    pt_d = nc.dram_tensor("pt", [NTK, E], MDT, kind=kin)
    wt_d = nc.dram_tensor("wt", [9, E, E], CDT, kind=kin)
    out_d = nc.dram_tensor("out", [E, PIX], F32, kind=kout)
    if not external_io:
        outs_d = nc.dram_tensor("outs", [1, 8], F32, kind="ExternalOutput")

    with tile.TileContext(nc) as tc, ExitStack() as ctx:
        consts = ctx.enter_context(tc.tile_pool(name="consts", bufs=1))
        pin = ctx.enter_context(tc.tile_pool(name="pin", bufs=6))
        psb = ctx.enter_context(tc.tile_pool(name="psb", bufs=6))
        pg_pool = ctx.enter_context(tc.tile_pool(name="pg", bufs=IR_MAX // 2 + 2))
        ppool = ctx.enter_context(
            tc.tile_pool(name="ppsum", bufs=8, space="PSUM"))
        pP = ctx.enter_context(tc.tile_pool(name="pP", bufs=2))
        pdram = ctx.enter_context(
            tc.tile_pool(name="pdram", bufs=8, space="DRAM"))
        pout = ctx.enter_context(tc.tile_pool(name="pout", bufs=2))

        # --- load constants once ---
        wa_sb = consts.tile([128, 14], MDT)
        nc.sync.dma_start(out=wa_sb[:], in_=wa_d[:])
        wb_sb = consts.tile([65, 14], MDT)
        nc.sync.dma_start(out=wb_sb[:], in_=wb_d[:])
        mz_sb = consts.tile([14, NTK], MDT)
        nc.sync.dma_start(out=mz_sb[:], in_=mz_d[:])
        bz_sb = consts.tile([NTK, 1], F32)
        nc.sync.dma_start(out=bz_sb[:], in_=bz_d[:])
        on48_sb = consts.tile([NTK, 1], MDT)
        nc.sync.dma_start(out=on48_sb[:], in_=on48_d[:])
        on1_sb = consts.tile([1, NTK], MDT)
        nc.sync.dma_start(out=on1_sb[:], in_=on1_d[:])
        pt_sb = consts.tile([NTK, E], MDT)
        nc.sync.dma_start(out=pt_sb[:], in_=pt_d[:])
        wt_sb = consts.tile([E, 9 * E], CDT)
        for t in range(9):
            nc.sync.dma_start(out=wt_sb[:, t * E:(t + 1) * E], in_=wt_d[t])

        n_strips = h // r_out
        conv_queue = deque()
        cur_out = {"t": None}
        last_exp_inst = None

        def emit_conv_pair(item):
            istt, yA, drain_sel = item
            cP3, cr0 = istt["P3"], istt["r0"]
            pcv = ppool.tile([128, 512], F32, tag="bank")
            taps = []
            for ky in (1, 0, 2):
                rlo, rhi = yA, yA + 1
                if yA + ky - 1 < 0:
                    rlo = yA + 1
                if yA + 1 + ky - 1 > h - 1:
                    rhi = yA
                for kx in (0, 1, 2):
                    taps.append((ky, kx, rlo, rhi))
            for ti, (ky, kx, rlo, rhi) in enumerate(taps):
                nr = rhi - rlo + 1
                lr = rlo + ky - 1 - cr0
                tap = ky * 3 + kx
                nc.tensor.matmul(
                    pcv[:, (rlo - yA) * w:(rhi - yA + 1) * w],
                    wt_sb[:, tap * E:(tap + 1) * E],
                    cP3[:, lr:lr + nr, kx:kx + w],
                    start=(ti == 0), stop=(ti == len(taps) - 1))
            # drain into the 8-row staging buffer; DMA when full
            q = (yA // 2) % 4
            if q == 0:
                cur_out["t"] = pout.tile([128, 2048], F32, tag="outsb",
                                         name="outsb")
            dst = cur_out["t"][:, q * 512:(q + 1) * 512]
            if drain_sel:
                nc.vector.tensor_copy(dst, pcv[:])
            else:
                nc.scalar.copy(dst, pcv[:])
            if q == 3:
                g0 = yA - 6
                nc.gpsimd.dma_start(out=out_d[:, g0 * w:(g0 + 8) * w],
                                      in_=cur_out["t"][:])

        def emit_body():
            nonlocal last_exp_inst
            last_exp_inst = None
            for s in range(n_strips):
                stt = make_strip(s)
                emit_phaseA(stt)
                emit_unified(stt, 0, stt["NCH"] + 4)
            while conv_queue:
                emit_conv_pair(conv_queue.popleft())

        def make_strip(s):
            y0, y1 = s * r_out, (s + 1) * r_out
            r0, r1 = max(0, y0 - 1), min(h - 1, y1)
            chunks = []
            r = r0
            while r <= r1:
                nrows = min(2, r1 - r + 1)
                chunks.append((r, nrows))
                r += nrows
            return {"s": s, "y0": y0, "y1": y1, "r0": r0, "r1": r1,
                    "ir": r1 - r0 + 1, "chunks": chunks, "NCH": len(chunks),
                    "P3": None, "g": [], "pzs": {}, "ps": {}, "wun": {},
                    "rr": {}, "rb": {}, "ppu": {}, "s3n": 0, "s0done": 0}

        def pair_ready(item):
            stt, yA, dsel = item
            p = (yA - stt["y0"]) // 2
            return stt["s3n"] >= min(p + 3, stt["NCH"])

        def emit_phaseA(stt):
            nonlocal last_exp_inst
            P_t = pP.tile([128, IR_MAX * PITCH], CDT, tag="P")
            P3 = P_t[:].rearrange("p (r c) -> p r c", c=PITCH)
            stt["P3"] = P3
            ir = stt["ir"]
            # zero spacer columns (left/right conv zero-padding)
            if conv_dt == "bf16":
                nc.vector.memset(P3[:, :ir, 0:1], 0.0)
                nc.vector.memset(P3[:, :ir, PITCH - 1:PITCH], 0.0)
            else:
                nc.vector.memset(P3[:, :ir, 0:1].bitcast(F32), 0.0)
                nc.vector.memset(
                    P3[:, :ir, PITCH - 1:PITCH].bitcast(F32), 0.0)
            for pi in range((stt["y1"] - stt["y0"]) // 2):
                conv_queue.append((stt, stt["y0"] + 2 * pi, pi % 2))
            first_gelu_inst = None
            for (r, nrows) in stt["chunks"]:
                npix = nrows * w
                off = r * w
                xf_t = pin.tile([128, 512], MDT, tag="xf")
                nc.sync.dma_start(out=xf_t[:, :npix],
                                  in_=xf_d[:, off:off + npix])
                fb_t = pin.tile([65, 512], MDT, tag="fb")
                nc.scalar.dma_start(out=fb_t[:, :npix],
                                    in_=fb_d[:, off:off + npix])
                pg = ppool.tile([14, 512], F32, tag="bank", name="pg")
                nc.tensor.matmul(pg[:, :npix], wa_sb[:], xf_t[:, :npix],
                                 start=True, stop=False)
                nc.tensor.matmul(pg[:, :npix], wb_sb[:], fb_t[:, :npix],
                                 start=False, stop=True)
                g_t = pg_pool.tile([14, 512], MDT, tag="g")
                inst = nc.scalar.activation(g_t[:, :npix], pg[:, :npix],
                                            gelu_fn)
                if first_gelu_inst is None:
                    first_gelu_inst = inst
                stt["g"].append(g_t)
                if conv_queue and pair_ready(conv_queue[0]):
                    emit_conv_pair(conv_queue.popleft())
            # keep ACT phases ordered across strips so walrus doesn't reload
            # activation tables on interleaved gelu/exp runs
            if (last_exp_inst is not None and first_gelu_inst is not None
                    and ACT_DEP):
                bass._add_dep_helper(first_gelu_inst.ins, last_exp_inst.ins,
                                     sync=True, reason="act-table-phase-order")

        def stage0(stt, ci):   # fc2+expand matmul, exp
            nonlocal last_exp_inst
            r, nrows = stt["chunks"][ci]
            npix = nrows * w
            pzs = ppool.tile([NTK, 512], F32, tag="bank", name="pzs")
            nc.tensor.matmul(pzs[0:NTK, :npix], mz_sb[:],
                             stt["g"][ci][:, :npix])
            wun = psb.tile([NTK, 512], MDT, tag="wun", name="wun")
            stt["wun"][ci] = wun
            last_exp_inst = nc.scalar.activation(
                wun[:, :npix], pzs[0:NTK, :npix], AFT.Exp, bias=bz_sb[:])
            stt["s0done"] = ci + 1

        def stage1(stt, ci):   # colsum, reciprocal
            r, nrows = stt["chunks"][ci]
            npix = nrows * w
            ps_t = ppool.tile([1, 512], F32, tag="bank", name="ps_t")
            stt["ps"][ci] = ps_t
            nc.tensor.matmul(ps_t[:, :npix], on48_sb[:],
                             stt["wun"][ci][:, :npix])
            rr = psb.tile([1, 512], F32, tag="rr", name="rr")
            stt["rr"][ci] = rr
            nc.vector.reciprocal_approx_fast(
                rr[:, :npix], stt["ps"][ci][:, :npix])

        def stage2(stt, ci):   # broadcast 1/S via DMA, prompt matmul on
            r, nrows = stt["chunks"][ci]     # unnormalized weights
            npix = nrows * w
            rrd = pdram.tile([1, 512], F32, tag="rrd", name="rrd")
            eng_a, eng_b = ((nc.sync, nc.scalar) if ci % 2 == 0
                            else (nc.scalar, nc.sync))
            eng_a.dma_start(out=rrd[0:1, :npix], in_=stt["rr"][ci][0:1, :npix])
            rb = psb.tile([128, 512], F32, tag="rb", name="rb")
            stt["rb"][ci] = rb
            rd_ap = rrd[0:1, :npix]
            bcast = bass.AP(tensor=rd_ap.tensor, offset=rd_ap.offset,
                            ap=[[0, 128]] + rd_ap.ap[1:])
            eng_b.dma_start(out=rb[:, :npix], in_=bcast)
            ppu = ppool.tile([128, 512], F32, tag="bank", name="ppu")
            stt["ppu"][ci] = ppu
            nc.tensor.matmul(ppu[:, :npix], pt_sb[:],
                             stt["wun"][ci][:, :npix])

        def stage3(stt, ci):   # fused normalize + copy into P strip
            r, nrows = stt["chunks"][ci]
            npix = nrows * w
            lr = r - stt["r0"]
            dst = stt["P3"][:, lr:lr + nrows, 1:1 + w]
            u3 = stt["ppu"][ci][:, :npix].rearrange("p (r c) -> p r c", c=w)
            rb3 = stt["rb"][ci][:, :npix].rearrange("p (r c) -> p r c", c=w)
            nc.vector.scalar_tensor_tensor(
                out=dst, in0=u3, scalar=1.0, in1=rb3,
                op0=mybir.AluOpType.mult, op1=mybir.AluOpType.mult)

        def emit_unified(stt, it0, it1):
            NCH = stt["NCH"]
            for it in range(it0, it1):
                if it < NCH and it >= stt["s0done"]:
                    stage0(stt, it)
                if 0 <= it - 1 < NCH:
                    stage1(stt, it - 1)
                if 0 <= it - 2 < NCH:
                    stage2(stt, it - 2)
                if 0 <= it - 3 < NCH:
                    stage3(stt, it - 3)
                    stt["s3n"] = it - 2
                while conv_queue:
                    if not pair_ready(conv_queue[0]):
                        break
                    if (len(conv_queue) <= 6
                            and stt["s"] < n_strips - 1):
                        break  # keep filler pairs for next strip's phase A
                    emit_conv_pair(conv_queue.popleft())
                    break  # at most one pair per iteration

        if repeat > 1:
            with tc.For_i(0, repeat, 1):
                emit_body()
        else:
            emit_body()
        if not external_io:
            nc.sync.dma_start(out=outs_d[:], in_=out_d[0:1, 0:8])

    nc.compile()
    return nc


_cache = {}

CONV_DT = None  # set to "bf16" to run the conv in bf16


def _wt_np_dtype():
    if CONV_DT == "bf16":
        import ml_dtypes
        return ml_dtypes.bfloat16
    return np.float32


def get_program(h=256, w=256, r_out=32, gelu_fn=AFT.Gelu):
    key = (h, w, r_out, gelu_fn, CONV_DT)
    if key not in _cache:
        _cache[key] = build_program(h, w, r_out, gelu_fn, conv_dt=CONV_DT)
    return _cache[key]


def make_weight_inputs(prompt, conv_w, b_fc1_w, b_fc1_b, b_fc2_w, b_fc2_b,
                       t_fc1_w, t_fc1_b, t_fc2_w, t_fc2_b):
    f = np.float32
    wa = np.zeros((128, 14), f)
    wa[:64, :8] = b_fc1_w.T
    wa[64:128, 8:14] = t_fc1_w[:, :64].T
    wb = np.zeros((65, 14), f)
    wb[:64, 8:14] = t_fc1_w[:, 64:].T
    wb[64, :8] = b_fc1_b
    wb[64, 8:14] = t_fc1_b
    mz = np.zeros((14, NTK), f)
    bz = np.zeros((NTK, 1), f)
    for t in range(NT):
        for k in range(NB):
            c = t * NB + k
            mz[:8, c] = b_fc2_w[k, :]
            mz[8:, c] = t_fc2_w[t, :]
            bz[c, 0] = b_fc2_b[k] + t_fc2_b[t]
    return {
        "wa": wa,
        "wb": wb,
        "mz": mz,
        "bz": bz,
        "on48": np.ones((NTK, 1), f),
        "on1": np.ones((1, NTK), f),
        "pt": np.ascontiguousarray(prompt.reshape(NTK, E).astype(f)),
        "wt": np.ascontiguousarray(
            conv_w.transpose(2, 3, 1, 0).reshape(9, E, E).astype(
                _wt_np_dtype())),
    }


def make_core_inputs(x_b, flux_b, weights, h, w):
    PIX = h * w
    f = np.float32
    xf = np.concatenate(
        [x_b.reshape(DIM, PIX), flux_b[:64].reshape(64, PIX)], axis=0)
    fb = np.concatenate(
        [flux_b[64:].reshape(64, PIX), np.ones((1, PIX), f)], axis=0)
    m = {"xf": np.ascontiguousarray(xf, dtype=f),
         "fb": np.ascontiguousarray(fb, dtype=f)}
    m.update(weights)
    return m


def kernel(x, flux, prompt, conv_w, b_fc1_w, b_fc1_b, b_fc2_w, b_fc2_b,
           t_fc1_w, t_fc1_b, t_fc2_w, t_fc2_b):
    x = np.asarray(x, np.float32)
    flux = np.asarray(flux, np.float32)
    flux = np.where(np.isnan(flux), np.float32(0), flux)
    h, w = x.shape[2], x.shape[3]

    nc = get_program(h=h, w=w)
    weights = make_weight_inputs(
        np.asarray(prompt, np.float32), np.asarray(conv_w, np.float32),
        np.asarray(b_fc1_w, np.float32), np.asarray(b_fc1_b, np.float32),
        np.asarray(b_fc2_w, np.float32), np.asarray(b_fc2_b, np.float32),
        np.asarray(t_fc1_w, np.float32), np.asarray(t_fc1_b, np.float32),
        np.asarray(t_fc2_w, np.float32), np.asarray(t_fc2_b, np.float32))
    in_maps = [make_core_inputs(x[i], flux[i], weights, h, w)
               for i in range(NCORES)]
    res = run_bass_kernel_spmd(nc, in_maps, list(range(NCORES)))
    out = np.stack([res.results[i]["out"].reshape(E, h, w)
                    for i in range(NCORES)], axis=0)
    return out
